# revision 19
# baseline (speedup 1.0000x reference)
"""Distributed Trainium2 (8-core) kernel for CausalSelfAttention.

Problem: B=2, T=2048, D=2048, NH=16 q-heads, NKV=4 kv-heads, HD=128.
  q,k,v projections -> RMSNorm(q,k) over head dim -> RoPE(q,k) -> q*gain
  -> v += ve_embed -> GQA causal softmax attention -> out proj Wo.

Sharding (8 cores = 2 batch groups x 4 tensor-parallel ranks):
  core (b*4 + c) handles batch b, q-heads [4c,4c+4), kv-head c.
  After attention each core holds yT_local [512, T] (feature-major).
  AllGather within the 4-rank group -> yT_full [2048, T]; each core then
  computes a disjoint 512-column slice of the output projection, so the
  host-side unshard is a pure concatenation.

Compute dtype: bf16 matmuls (f32 PSUM accumulate), f32 softmax/norm math.
Softmax runs without max-subtraction: rms-normed q,k bound |score| by
sqrt(HD) ~= 11.3, so exp() cannot overflow fp32/bf16.
"""

import sys

if "/opt/trn_rl_repo" not in sys.path:
    sys.path.insert(0, "/opt/trn_rl_repo")

from contextlib import ExitStack

import ml_dtypes
import numpy as np

import concourse.bass as bass
import concourse.mybir as mybir
import concourse.tile as tile
from concourse import bacc
from concourse.bass_utils import run_bass_kernel_spmd

BF16 = mybir.dt.bfloat16
F32 = mybir.dt.float32
NPBF16 = ml_dtypes.bfloat16

B, T, D = 2, 2048, 2048
NH, NKV, HD = 16, 4, 128
HPC = NH // NKV          # q-heads per core = 4
QF = HPC * HD            # 512 q features per core
ROPE_BASE = 10000.0
EPS = 1.1920929e-07
NT = T // 128            # 16 token tiles
ND = D // 128            # 16 contraction tiles
NB = T // 512            # 4 tq blocks of 512
NCORES = 8
GROUPS = [[0, 1, 2, 3], [4, 5, 6, 7]]


def _emit(tc, ctx):
    nc = tc.nc  # pools are entered on ctx / with-blocks inside the TileContext

    # ---- DRAM I/O ----
    xt_d = nc.dram_tensor("xt", [D, T], BF16, kind="ExternalInput").ap()
    wq_d = nc.dram_tensor("wq", [D, QF], BF16, kind="ExternalInput").ap()
    wkv_d = nc.dram_tensor("wkv", [D, 2 * HD], BF16, kind="ExternalInput").ap()
    ve_d = nc.dram_tensor("ve", [T, HD], F32, kind="ExternalInput").ap()
    wo_d = nc.dram_tensor("wo", [D, QF], BF16, kind="ExternalInput").ap()
    cs_d = nc.dram_tensor("cs", [T, 512], BF16, kind="ExternalInput").ap()
    qg_d = nc.dram_tensor("qg", [1, HPC], F32, kind="ExternalInput").ap()
    mask_d = nc.dram_tensor("mask", [128, 128], BF16, kind="ExternalInput").ap()
    id_d = nc.dram_tensor("ident", [128, 128], BF16, kind="ExternalInput").ap()
    out_d = nc.dram_tensor("out", [T, QF], F32, kind="ExternalOutput").ap()

    ag_in = [nc.dram_tensor(f"ag_in{j}", [QF, 512], BF16) for j in range(NB)]
    ag_out = [nc.dram_tensor(f"ag_out{j}", [D, 512], BF16) for j in range(NB)]

    # ---- persistent SBUF ----
    persist = ctx.enter_context(tc.tile_pool(name="persist", bufs=1))
    qT_all = persist.tile([128, NT * 512], BF16, tag="qTall", name="qTall")
    kT = persist.tile([128, T], BF16, tag="kT", name="kT")
    v_sb = [persist.tile([128, HD + 1], BF16, tag=f"v{t}", name=f"v{t}") for t in range(NT)]
    mask_sb = persist.tile([128, 128], BF16, tag="mask", name="mask")
    qg_bc = persist.tile([128, HPC], F32, tag="qgbc", name="qgbc")
    id_sb = persist.tile([128, 128], BF16, tag="ident", name="ident")
    xt_sb = [persist.tile([128, T], BF16, tag=f"xt{d}", name=f"xt{d}") for d in range(ND)]
    wq_sb = [persist.tile([128, QF], BF16, tag=f"wq{d}", name=f"wq{d}") for d in range(ND)]
    cs_sb = [persist.tile([128, 512], BF16, tag=f"cs{t}", name=f"cs{t}") for t in range(NT)]

    nc.gpsimd.dma_start(id_sb[:], id_d[:, :])
    nc.gpsimd.dma_start(mask_sb[:], mask_d[:, :])

    # ---- phase K: kv projection for all t ----
    with (
        tc.tile_pool(name="pkw", bufs=1) as pkw,
        tc.tile_pool(name="pks", bufs=3) as pks,
        tc.tile_pool(name="pkps", bufs=2, space="PSUM") as pkps,
    ):
        wkv_sb = [pkw.tile([128, 2 * HD], BF16, tag=f"wkv{d}", name=f"wkv{d}") for d in range(ND)]
        ve_sb = [pkw.tile([128, HD], F32, tag=f"ve{t}", name=f"ve{t}") for t in range(NT)]
        ones_sb = pkw.tile([1, 128], F32, tag="ones")
        qg_sb = pkw.tile([1, HPC], F32, tag="qg")

        for d in range(ND):
            nc.sync.dma_start(wkv_sb[d][:], wkv_d[128 * d : 128 * (d + 1), :])
        nc.sync.dma_start(qg_sb[:], qg_d[:, :])
        # x chunks: first column block on sync to unblock t=0 fast, rest split
        for cj in range(4):
            for d in range(ND):
                eng = nc.scalar if (d + cj) % 2 == 0 else nc.sync
                eng.dma_start(
                    xt_sb[d][:, 512 * cj : 512 * (cj + 1)],
                    xt_d[128 * d : 128 * (d + 1), 512 * cj : 512 * (cj + 1)],
                )
        for d in range(ND):
            nc.gpsimd.dma_start(wq_sb[d][:], wq_d[128 * d : 128 * (d + 1), :])
        for t in range(NT):
            nc.gpsimd.dma_start(ve_sb[t][:], ve_d[128 * t : 128 * (t + 1), :])
            nc.gpsimd.dma_start(cs_sb[t][:], cs_d[128 * t : 128 * (t + 1), :])

        # broadcast q_gain across partitions, fold in 1/sqrt(HD)
        nc.vector.memset(ones_sb[:], 1.0)
        qg_ps = pkps.tile([128, HPC], F32, tag="qgps")
        nc.tensor.matmul(qg_ps[:], ones_sb[:], qg_sb[:], start=True, stop=True)
        nc.scalar.mul(qg_bc[:], qg_ps[:], 1.0 / float(np.sqrt(HD)))

        for t in range(NT):
            kv_ps = pkps.tile([128, 2 * HD], F32, tag="kvps")
            for d in range(ND):
                nc.tensor.matmul(
                    kv_ps[:], xt_sb[d][:, 128 * t : 128 * (t + 1)], wkv_sb[d][:],
                    start=(d == 0), stop=(d == ND - 1),
                )
            sqk = pks.tile([128, HD], F32, tag="sqk")
            ssqk = pks.tile([128, 1], F32, tag="ssqk")
            nc.scalar.activation(
                sqk[:], kv_ps[:, 0:HD],
                mybir.ActivationFunctionType.Square, accum_out=ssqk[:],
            )
            nc.vector.tensor_scalar(
                ssqk[:], ssqk[:], 1.0 / HD, EPS,
                mybir.AluOpType.mult, mybir.AluOpType.add,
            )
            nc.vector.reciprocal(ssqk[:], ssqk[:])
            rk = pks.tile([128, 1], F32, tag="rk")
            nc.scalar.sqrt(rk[:], ssqk[:])
            kn = pks.tile([128, HD], BF16, tag="kn")
            nc.vector.tensor_scalar(
                kn[:], kv_ps[:, 0:HD], rk[:], None, mybir.AluOpType.mult
            )
            k_ro = pks.tile([128, HD], BF16, tag="kro")
            tmk = pks.tile([128, HD], BF16, tag="tmk")
            co, si = cs_sb[t][:, 0:64], cs_sb[t][:, 256:320]
            x1, x2 = kn[:, 0:64], kn[:, 64:128]
            nc.vector.tensor_mul(tmk[:, 0:64], x1, co)
            nc.vector.tensor_mul(tmk[:, 64:128], x2, si)
            nc.vector.tensor_sub(k_ro[:, 0:64], tmk[:, 0:64], tmk[:, 64:128])
            nc.vector.tensor_mul(tmk[:, 0:64], x1, si)
            nc.vector.tensor_mul(tmk[:, 64:128], x2, co)
            nc.vector.tensor_add(k_ro[:, 64:128], tmk[:, 0:64], tmk[:, 64:128])
            nc.sync.dma_start_transpose(kT[:, 128 * t : 128 * (t + 1)], k_ro[:])
            nc.vector.tensor_add(v_sb[t][:, 0:HD], kv_ps[:, HD : 2 * HD], ve_sb[t][:])
            nc.vector.memset(v_sb[t][:, HD : HD + 1], 1.0)

    # ---- main section: per tq-block j: q-proj + attention + AllGather ----
    pw = ctx.enter_context(tc.tile_pool(name="pw", bufs=1))
    wo_sb = [pw.tile([128, QF], BF16, tag=f"wo{f}", name=f"wo{f}") for f in range(ND)]
    for f in range(ND):
        nc.gpsimd.dma_start(wo_sb[f][:], wo_d[128 * f : 128 * (f + 1), :])
    with (
        tc.tile_pool(name="pms", bufs=3) as pms,
        tc.tile_pool(name="pt_pool", bufs=6) as ptp,
        tc.tile_pool(name="pqps", bufs=2, space="PSUM") as pqps,
        tc.tile_pool(name="psps", bufs=2, space="PSUM") as psps,
        tc.tile_pool(name="paps", bufs=4, space="PSUM") as paps,
    ):
        qT_v = qT_all[:, :].rearrange("p (t x) -> p t x", x=512)
        for j in range(NB):
            for t in range(4 * j, 4 * j + 4):
                q_ps = pqps.tile([128, QF], F32, tag="qps")
                for d in range(ND):
                    nc.tensor.matmul(
                        q_ps[:], xt_sb[d][:, 128 * t : 128 * (t + 1)], wq_sb[d][:],
                        start=(d == 0), stop=(d == ND - 1),
                    )
                sq = pms.tile([128, HD], F32, tag="sq")
                ssq = pms.tile([128, HPC], F32, tag="ssq")
                for h in range(HPC):
                    nc.scalar.activation(
                        sq[:], q_ps[:, HD * h : HD * (h + 1)],
                        mybir.ActivationFunctionType.Square,
                        accum_out=ssq[:, h : h + 1],
                    )
                nc.vector.tensor_scalar(
                    ssq[:], ssq[:], 1.0 / HD, EPS,
                    mybir.AluOpType.mult, mybir.AluOpType.add,
                )
                nc.vector.reciprocal(ssq[:], ssq[:])
                rinv = pms.tile([128, HPC], F32, tag="rinv")
                nc.scalar.sqrt(rinv[:], ssq[:])
                qn = pms.tile([128, QF], BF16, tag="qn")
                for h in range(HPC):
                    nc.vector.tensor_scalar(
                        qn[:, HD * h : HD * (h + 1)], q_ps[:, HD * h : HD * (h + 1)],
                        rinv[:, h : h + 1], qg_bc[:, h : h + 1],
                        mybir.AluOpType.mult, mybir.AluOpType.mult,
                    )
                co4 = cs_sb[t][:, 0:256].rearrange("p (h x) -> p h x", h=HPC)
                si4 = cs_sb[t][:, 256:512].rearrange("p (h x) -> p h x", h=HPC)
                q_ro = pms.tile([128, QF], BF16, tag="qro")
                tma = pms.tile([128, 256], BF16, tag="ropetma")
                tmb = pms.tile([128, 256], BF16, tag="ropetmb")
                qn_v = qn[:, :].rearrange("p (h two x) -> p h two x", h=HPC, two=2)
                qro_v = q_ro[:, :].rearrange("p (h two x) -> p h two x", h=HPC, two=2)
                q1, q2 = qn_v[:, :, 0, :], qn_v[:, :, 1, :]
                tma_v = tma[:, :].rearrange("p (h x) -> p h x", h=HPC)
                tmb_v = tmb[:, :].rearrange("p (h x) -> p h x", h=HPC)
                nc.vector.tensor_mul(tma_v, q1, co4)
                nc.vector.tensor_mul(tmb_v, q2, si4)
                nc.vector.tensor_sub(qro_v[:, :, 0, :], tma_v, tmb_v)
                nc.vector.tensor_mul(tma_v, q1, si4)
                nc.vector.tensor_mul(tmb_v, q2, co4)
                nc.vector.tensor_add(qro_v[:, :, 1, :], tma_v, tmb_v)
                qdst = qT_all[:, 512 * t : 512 * (t + 1)].rearrange(
                    "p (h x) -> p h x", h=HPC
                )
                nc.sync.dma_start_transpose(qdst, q_ro[:])

            # attention for this tq block, all heads
            yblk = []
            for h in range(HPC):
                ybk = ptp.tile([128, 512], BF16, tag=f"yblk{h}", name=f"yblk{h}", bufs=2)
                yblk.append(ybk)
                ntk = 4 * j + 4
                av = [paps.tile([128, HD + 1], F32, tag="av", name="av") for _ in range(4)]
                for i in range(ntk):
                    m_lo = max(0, i - 4 * j)   # first valid tq chunk of this block
                    ncols = 128 * (4 - m_lo)
                    s_ps = psps.tile([128, 512], F32, tag="sps")
                    nc.tensor.matmul(
                        s_ps[:, 0:ncols],
                        kT[:, 128 * i : 128 * (i + 1)],
                        qT_v[:, 4 * j + m_lo : 4 * j + 4, HD * h : HD * (h + 1)],
                        start=True, stop=True,
                    )
                    pt = ptp.tile([128, 512], BF16, tag="pt")
                    nc.scalar.activation(
                        pt[:, 0:ncols], s_ps[:, 0:ncols],
                        mybir.ActivationFunctionType.Exp,
                    )
                    if i >= 4 * j:  # partial (true diagonal) chunk
                        nc.vector.tensor_mul(pt[:, 0:128], pt[:, 0:128], mask_sb[:])
                    for m in range(m_lo, 4):
                        nc.tensor.matmul(
                            av[m][:], pt[:, 128 * (m - m_lo) : 128 * (m - m_lo + 1)],
                            v_sb[i][:],
                            start=(i == 0), stop=(i == 4 * j + m),
                        )
                for m in range(4):
                    t_abs = 4 * j + m
                    rs = ptp.tile([128, 1], F32, tag="rs")
                    nc.vector.reciprocal(rs[:], av[m][:, HD : HD + 1])
                    y_tok = ptp.tile([128, HD], BF16, tag="ytok")
                    nc.vector.tensor_scalar(
                        y_tok[:], av[m][:, 0:HD], rs[:], None, mybir.AluOpType.mult
                    )
                    yt_ps = psps.tile([128, HD], BF16, tag="sps")
                    nc.tensor.transpose(yt_ps[:], y_tok[:], id_sb[:])
                    nc.vector.tensor_copy(yblk[h][:, 128 * m : 128 * (m + 1)], yt_ps[:])

            # ship block j through its own AllGather; overlaps block j+1
            for h in range(HPC):
                nc.sync.dma_start(
                    ag_in[j].ap()[128 * h : 128 * (h + 1), :], yblk[h][:]
                )
            nc.gpsimd.collective_compute(
                "AllGather",
                mybir.AluOpType.bypass,
                replica_groups=GROUPS,
                ins=[ag_in[j].ap().opt()],
                outs=[ag_out[j].ap().opt()],
            )

    # ---- out projection (yf blocks land in j order; only AG3 exposed) ----
    with (
        tc.tile_pool(name="p4w", bufs=1) as p4w,
        tc.tile_pool(name="p4s", bufs=3) as p4s,
        tc.tile_pool(name="p4ps", bufs=8, space="PSUM") as p4ps,
    ):
        yf_sb = [p4w.tile([128, T], BF16, tag=f"yf{f}", name=f"yf{f}") for f in range(ND)]
        for j in range(NB):
            for f in range(ND):
                eng = nc.sync if f % 2 == 0 else nc.scalar
                eng.dma_start(
                    yf_sb[f][:, 512 * j : 512 * (j + 1)],
                    ag_out[j].ap()[128 * f : 128 * (f + 1), :],
                )
        for t in range(NT):
            o_ps = p4ps.tile([128, QF], F32, tag="ops")
            for f in range(ND):
                nc.tensor.matmul(
                    o_ps[:], yf_sb[f][:, 128 * t : 128 * (t + 1)], wo_sb[f][:],
                    start=(f == 0), stop=(f == ND - 1),
                )
            o_sb = p4s.tile([128, QF], F32, tag="osb")
            nc.scalar.copy(o_sb[:], o_ps[:])
            nc.sync.dma_start(out_d[128 * t : 128 * (t + 1), :], o_sb[:])


_CACHED = None


def _build():
    global _CACHED
    if _CACHED is None:
        nc = bacc.Bacc(
            "TRN2", target_bir_lowering=False, debug=False, num_devices=NCORES
        )
        with tile.TileContext(nc) as tc:
            with ExitStack() as ctx:
                _emit(tc, ctx)
        nc.compile()
        _CACHED = nc
    return _CACHED


def _in_maps(x, ve_embed, Wq, Wk, Wv, Wo, q_gain):
    x = np.asarray(x, np.float32)
    ve_embed = np.asarray(ve_embed, np.float32)
    Wq = np.asarray(Wq, np.float32)
    Wk = np.asarray(Wk, np.float32)
    Wv = np.asarray(Wv, np.float32)
    Wo = np.asarray(Wo, np.float32)
    q_gain = np.asarray(q_gain, np.float32)

    tt = np.arange(T, dtype=np.float32)
    inv_freq = (
        1.0 / (ROPE_BASE ** (np.arange(0, HD, 2, dtype=np.float32) / np.float32(HD)))
    ).astype(np.float32)
    f = np.outer(tt, inv_freq)
    cs = np.concatenate(
        [np.tile(np.cos(f), (1, 4)), np.tile(np.sin(f), (1, 4))], axis=1
    ).astype(NPBF16)

    p = np.arange(128)[:, None]
    w = np.arange(128)[None, :]
    mask = (w >= p).astype(NPBF16)

    maps = []
    for core in range(NCORES):
        b, c = divmod(core, 4)
        qrows = slice(QF * c, QF * (c + 1))
        krows = slice(HD * c, HD * (c + 1))
        maps.append(
            {
                "xt": np.ascontiguousarray(x[b].T).astype(NPBF16),
                "wq": np.ascontiguousarray(Wq[qrows, :].T).astype(NPBF16),
                "wkv": np.ascontiguousarray(
                    np.concatenate([Wk[krows, :], Wv[krows, :]], axis=0).T
                ).astype(NPBF16),
                "ve": np.ascontiguousarray(ve_embed[b][:, krows]),
                "wo": np.ascontiguousarray(Wo[qrows, :].T).astype(NPBF16),
                "cs": cs,
                "qg": q_gain[None, HPC * c : HPC * (c + 1)].copy(),
                "mask": mask,
                "ident": np.eye(128, dtype=NPBF16),
            }
        )
    return maps


def _assemble(results):
    out = np.empty((B, T, D), np.float32)
    for core in range(NCORES):
        b, c = divmod(core, 4)
        out[b][:, QF * c : QF * (c + 1)] = results[core]["out"]
    return out


def run_traced(**inputs):
    nc = _build()
    maps = _in_maps(**inputs)
    r = run_bass_kernel_spmd(nc, maps, core_ids=list(range(NCORES)), trace=True)
    return _assemble(r.results), r


def kernel(**inputs):
    nc = _build()
    maps = _in_maps(**inputs)
    r = run_bass_kernel_spmd(nc, maps, core_ids=list(range(NCORES)))
    return _assemble(r.results)


# revision 20
# speedup vs baseline: 1.1805x; 1.1805x over previous
"""Distributed Trainium2 (8-core) kernel for CausalSelfAttention.

Problem: B=2, T=2048, D=2048, NH=16 q-heads, NKV=4 kv-heads, HD=128.
  q,k,v projections -> RMSNorm(q,k) over head dim -> RoPE(q,k) -> q*gain
  -> v += ve_embed -> GQA causal softmax attention -> out proj Wo.

Sharding (8 cores = 2 batch groups x 4 tensor-parallel ranks):
  core (b*4 + c) handles batch b, q-heads [4c,4c+4), kv-head c.
  After attention each core holds yT_local [512, T] (feature-major).
  AllGather within the 4-rank group -> yT_full [2048, T]; each core then
  computes a disjoint 512-column slice of the output projection, so the
  host-side unshard is a pure concatenation.

Compute dtype: bf16 matmuls (f32 PSUM accumulate), f32 softmax/norm math.
Softmax runs without max-subtraction: rms-normed q,k bound |score| by
sqrt(HD) ~= 11.3, so exp() cannot overflow fp32/bf16.
"""

import sys

if "/opt/trn_rl_repo" not in sys.path:
    sys.path.insert(0, "/opt/trn_rl_repo")

from contextlib import ExitStack

import ml_dtypes
import numpy as np

import concourse.bass as bass
import concourse.mybir as mybir
import concourse.tile as tile
from concourse import bacc
from concourse.bass_utils import run_bass_kernel_spmd

BF16 = mybir.dt.bfloat16
F32 = mybir.dt.float32
NPBF16 = ml_dtypes.bfloat16

B, T, D = 2, 2048, 2048
NH, NKV, HD = 16, 4, 128
HPC = NH // NKV          # q-heads per core = 4
QF = HPC * HD            # 512 q features per core
ROPE_BASE = 10000.0
EPS = 1.1920929e-07
NT = T // 128            # 16 token tiles
ND = D // 128            # 16 contraction tiles
NB = T // 512            # 4 tq blocks of 512
NCORES = 8
GROUPS = [[0, 1, 2, 3], [4, 5, 6, 7]]


def _emit(tc, ctx):
    nc = tc.nc  # pools are entered on ctx / with-blocks inside the TileContext

    # ---- DRAM I/O ----
    xt_d = nc.dram_tensor("xt", [D, T], BF16, kind="ExternalInput").ap()
    wq_d = nc.dram_tensor("wq", [D, QF], BF16, kind="ExternalInput").ap()
    wkv_d = nc.dram_tensor("wkv", [D, 2 * HD], BF16, kind="ExternalInput").ap()
    ve_d = nc.dram_tensor("ve", [T, HD], F32, kind="ExternalInput").ap()
    wo_d = nc.dram_tensor("wo", [D, QF], BF16, kind="ExternalInput").ap()
    cs_d = nc.dram_tensor("cs", [T, 512], BF16, kind="ExternalInput").ap()
    qg_d = nc.dram_tensor("qg", [1, HPC], F32, kind="ExternalInput").ap()
    mask_d = nc.dram_tensor("mask", [128, 128], BF16, kind="ExternalInput").ap()
    id_d = nc.dram_tensor("ident", [128, 128], BF16, kind="ExternalInput").ap()
    out_d = nc.dram_tensor("out", [T, QF], F32, kind="ExternalOutput").ap()

    ag_in = [nc.dram_tensor(f"ag_in{j}", [QF, 512], BF16) for j in range(NB)]
    ag_out = [nc.dram_tensor(f"ag_out{j}", [D, 512], BF16) for j in range(NB)]

    # ---- persistent SBUF ----
    persist = ctx.enter_context(tc.tile_pool(name="persist", bufs=1))
    qT_all = persist.tile([128, NT * 512], BF16, tag="qTall", name="qTall")
    kT = persist.tile([128, T], BF16, tag="kT", name="kT")
    v_sb = [persist.tile([128, HD + 1], BF16, tag=f"v{t}", name=f"v{t}") for t in range(NT)]
    mask_sb = persist.tile([128, 128], BF16, tag="mask", name="mask")
    qg_bc = persist.tile([128, HPC], F32, tag="qgbc", name="qgbc")
    id_sb = persist.tile([128, 128], BF16, tag="ident", name="ident")
    xt_sb = [persist.tile([128, T], BF16, tag=f"xt{d}", name=f"xt{d}") for d in range(ND)]
    wq_sb = [persist.tile([128, QF], BF16, tag=f"wq{d}", name=f"wq{d}") for d in range(ND)]
    cs_sb = [persist.tile([128, 512], BF16, tag=f"cs{t}", name=f"cs{t}") for t in range(NT)]

    nc.gpsimd.dma_start(id_sb[:], id_d[:, :])
    nc.gpsimd.dma_start(mask_sb[:], mask_d[:, :])

    # ---- phase K: kv projection for all t ----
    with (
        tc.tile_pool(name="pkw", bufs=1) as pkw,
        tc.tile_pool(name="pks", bufs=3) as pks,
        tc.tile_pool(name="pkps", bufs=2, space="PSUM") as pkps,
    ):
        wkv_sb = [pkw.tile([128, 2 * HD], BF16, tag=f"wkv{d}", name=f"wkv{d}") for d in range(ND)]
        ve_sb = [pkw.tile([128, HD], F32, tag=f"ve{t}", name=f"ve{t}") for t in range(NT)]
        ones_sb = pkw.tile([1, 128], F32, tag="ones")
        qg_sb = pkw.tile([1, HPC], F32, tag="qg")

        for d in range(ND):
            nc.sync.dma_start(wkv_sb[d][:], wkv_d[128 * d : 128 * (d + 1), :])
        nc.sync.dma_start(qg_sb[:], qg_d[:, :])
        # x chunks: first column block on sync to unblock t=0 fast, rest split
        for cj in range(4):
            for d in range(ND):
                eng = nc.scalar if (d + cj) % 2 == 0 else nc.sync
                eng.dma_start(
                    xt_sb[d][:, 512 * cj : 512 * (cj + 1)],
                    xt_d[128 * d : 128 * (d + 1), 512 * cj : 512 * (cj + 1)],
                )
        for d in range(ND):
            nc.gpsimd.dma_start(wq_sb[d][:], wq_d[128 * d : 128 * (d + 1), :])
        for t in range(NT):
            nc.gpsimd.dma_start(ve_sb[t][:], ve_d[128 * t : 128 * (t + 1), :])
            nc.gpsimd.dma_start(cs_sb[t][:], cs_d[128 * t : 128 * (t + 1), :])

        # broadcast q_gain across partitions, fold in 1/sqrt(HD)
        nc.vector.memset(ones_sb[:], 1.0)
        qg_ps = pkps.tile([128, HPC], F32, tag="qgps")
        nc.tensor.matmul(qg_ps[:], ones_sb[:], qg_sb[:], start=True, stop=True)
        nc.scalar.mul(qg_bc[:], qg_ps[:], 1.0 / float(np.sqrt(HD)))

        for t in range(NT):
            kv_ps = pkps.tile([128, 2 * HD], F32, tag="kvps")
            for d in range(ND):
                nc.tensor.matmul(
                    kv_ps[:], xt_sb[d][:, 128 * t : 128 * (t + 1)], wkv_sb[d][:],
                    start=(d == 0), stop=(d == ND - 1),
                )
            sqk = pks.tile([128, HD], F32, tag="sqk")
            ssqk = pks.tile([128, 1], F32, tag="ssqk")
            nc.scalar.activation(
                sqk[:], kv_ps[:, 0:HD],
                mybir.ActivationFunctionType.Square, accum_out=ssqk[:],
            )
            nc.vector.tensor_scalar(
                ssqk[:], ssqk[:], 1.0 / HD, EPS,
                mybir.AluOpType.mult, mybir.AluOpType.add,
            )
            nc.vector.reciprocal(ssqk[:], ssqk[:])
            rk = pks.tile([128, 1], F32, tag="rk")
            nc.scalar.sqrt(rk[:], ssqk[:])
            kn = pks.tile([128, HD], BF16, tag="kn")
            nc.vector.tensor_scalar(
                kn[:], kv_ps[:, 0:HD], rk[:], None, mybir.AluOpType.mult
            )
            k_ro = pks.tile([128, HD], BF16, tag="kro")
            tmk = pks.tile([128, HD], BF16, tag="tmk")
            co, si = cs_sb[t][:, 0:64], cs_sb[t][:, 256:320]
            x1, x2 = kn[:, 0:64], kn[:, 64:128]
            nc.vector.tensor_mul(tmk[:, 0:64], x1, co)
            nc.vector.tensor_mul(tmk[:, 64:128], x2, si)
            nc.vector.tensor_sub(k_ro[:, 0:64], tmk[:, 0:64], tmk[:, 64:128])
            nc.vector.tensor_mul(tmk[:, 0:64], x1, si)
            nc.vector.tensor_mul(tmk[:, 64:128], x2, co)
            nc.vector.tensor_add(k_ro[:, 64:128], tmk[:, 0:64], tmk[:, 64:128])
            nc.sync.dma_start_transpose(kT[:, 128 * t : 128 * (t + 1)], k_ro[:])
            nc.vector.tensor_add(v_sb[t][:, 0:HD], kv_ps[:, HD : 2 * HD], ve_sb[t][:])
            nc.vector.memset(v_sb[t][:, HD : HD + 1], 1.0)

    # ---- main section: per tq-block j: q-proj + attention + AllGather ----
    pw = ctx.enter_context(tc.tile_pool(name="pw", bufs=1))
    wo_sb = [pw.tile([128, QF], BF16, tag=f"wo{f}", name=f"wo{f}") for f in range(ND)]
    for f in range(ND):
        nc.gpsimd.dma_start(wo_sb[f][:], wo_d[128 * f : 128 * (f + 1), :])
    with (
        tc.tile_pool(name="pms", bufs=3) as pms,
        tc.tile_pool(name="pt_pool", bufs=6) as ptp,
        tc.tile_pool(name="pqps", bufs=2, space="PSUM") as pqps,
        tc.tile_pool(name="psps", bufs=2, space="PSUM") as psps,
        tc.tile_pool(name="paps", bufs=4, space="PSUM") as paps,
    ):
        qT_v = qT_all[:, :].rearrange("p (t x) -> p t x", x=512)
        for j in range(NB):
            for t in range(4 * j, 4 * j + 4):
                q_ps = pqps.tile([128, QF], F32, tag="qps")
                for d in range(ND):
                    nc.tensor.matmul(
                        q_ps[:], xt_sb[d][:, 128 * t : 128 * (t + 1)], wq_sb[d][:],
                        start=(d == 0), stop=(d == ND - 1),
                    )
                sq = pms.tile([128, HD], F32, tag="sq")
                ssq = pms.tile([128, HPC], F32, tag="ssq")
                for h in range(HPC):
                    nc.scalar.activation(
                        sq[:], q_ps[:, HD * h : HD * (h + 1)],
                        mybir.ActivationFunctionType.Square,
                        accum_out=ssq[:, h : h + 1],
                    )
                nc.vector.tensor_scalar(
                    ssq[:], ssq[:], 1.0 / HD, EPS,
                    mybir.AluOpType.mult, mybir.AluOpType.add,
                )
                nc.vector.reciprocal(ssq[:], ssq[:])
                rinv = pms.tile([128, HPC], F32, tag="rinv")
                nc.scalar.sqrt(rinv[:], ssq[:])
                qn = pms.tile([128, QF], BF16, tag="qn")
                for h in range(HPC):
                    nc.vector.tensor_scalar(
                        qn[:, HD * h : HD * (h + 1)], q_ps[:, HD * h : HD * (h + 1)],
                        rinv[:, h : h + 1], qg_bc[:, h : h + 1],
                        mybir.AluOpType.mult, mybir.AluOpType.mult,
                    )
                co4 = cs_sb[t][:, 0:256].rearrange("p (h x) -> p h x", h=HPC)
                si4 = cs_sb[t][:, 256:512].rearrange("p (h x) -> p h x", h=HPC)
                q_ro = pms.tile([128, QF], BF16, tag="qro")
                tma = pms.tile([128, 256], BF16, tag="ropetma")
                tmb = pms.tile([128, 256], BF16, tag="ropetmb")
                qn_v = qn[:, :].rearrange("p (h two x) -> p h two x", h=HPC, two=2)
                qro_v = q_ro[:, :].rearrange("p (h two x) -> p h two x", h=HPC, two=2)
                q1, q2 = qn_v[:, :, 0, :], qn_v[:, :, 1, :]
                tma_v = tma[:, :].rearrange("p (h x) -> p h x", h=HPC)
                tmb_v = tmb[:, :].rearrange("p (h x) -> p h x", h=HPC)
                nc.vector.tensor_mul(tma_v, q1, co4)
                nc.vector.tensor_mul(tmb_v, q2, si4)
                nc.vector.tensor_sub(qro_v[:, :, 0, :], tma_v, tmb_v)
                nc.vector.tensor_mul(tma_v, q1, si4)
                nc.vector.tensor_mul(tmb_v, q2, co4)
                nc.vector.tensor_add(qro_v[:, :, 1, :], tma_v, tmb_v)
                for h in range(HPC):
                    qt_ps = psps.tile([128, HD], BF16, tag="sps")
                    nc.tensor.transpose(
                        qt_ps[:], q_ro[:, HD * h : HD * (h + 1)], id_sb[:]
                    )
                    nc.vector.tensor_copy(
                        qT_all[:, 512 * t + HD * h : 512 * t + HD * (h + 1)],
                        qt_ps[:],
                    )

            # attention for this tq block, all heads
            yblk = []
            for h in range(HPC):
                ybk = ptp.tile([128, 512], BF16, tag=f"yblk{h}", name=f"yblk{h}", bufs=2)
                yblk.append(ybk)
                ntk = 4 * j + 4
                av = [paps.tile([128, HD + 1], F32, tag="av", name="av") for _ in range(4)]
                for i in range(ntk):
                    m_lo = max(0, i - 4 * j)   # first valid tq chunk of this block
                    ncols = 128 * (4 - m_lo)
                    s_ps = psps.tile([128, 512], F32, tag="sps")
                    nc.tensor.matmul(
                        s_ps[:, 0:ncols],
                        kT[:, 128 * i : 128 * (i + 1)],
                        qT_v[:, 4 * j + m_lo : 4 * j + 4, HD * h : HD * (h + 1)],
                        start=True, stop=True,
                    )
                    pt = ptp.tile([128, 512], BF16, tag="pt")
                    nc.scalar.activation(
                        pt[:, 0:ncols], s_ps[:, 0:ncols],
                        mybir.ActivationFunctionType.Exp,
                    )
                    if i >= 4 * j:  # partial (true diagonal) chunk
                        nc.vector.tensor_mul(pt[:, 0:128], pt[:, 0:128], mask_sb[:])
                    for m in range(m_lo, 4):
                        nc.tensor.matmul(
                            av[m][:], pt[:, 128 * (m - m_lo) : 128 * (m - m_lo + 1)],
                            v_sb[i][:],
                            start=(i == 0), stop=(i == 4 * j + m),
                        )
                for m in range(4):
                    t_abs = 4 * j + m
                    rs = ptp.tile([128, 1], F32, tag="rs")
                    nc.vector.reciprocal(rs[:], av[m][:, HD : HD + 1])
                    y_tok = ptp.tile([128, HD], BF16, tag="ytok")
                    nc.vector.tensor_scalar(
                        y_tok[:], av[m][:, 0:HD], rs[:], None, mybir.AluOpType.mult
                    )
                    yt_ps = psps.tile([128, HD], BF16, tag="sps")
                    nc.tensor.transpose(yt_ps[:], y_tok[:], id_sb[:])
                    nc.vector.tensor_copy(yblk[h][:, 128 * m : 128 * (m + 1)], yt_ps[:])

            # ship block j through its own AllGather; overlaps block j+1
            for h in range(HPC):
                nc.sync.dma_start(
                    ag_in[j].ap()[128 * h : 128 * (h + 1), :], yblk[h][:]
                )
            nc.gpsimd.collective_compute(
                "AllGather",
                mybir.AluOpType.bypass,
                replica_groups=GROUPS,
                ins=[ag_in[j].ap().opt()],
                outs=[ag_out[j].ap().opt()],
            )

    # ---- out projection (yf blocks land in j order; only AG3 exposed) ----
    with (
        tc.tile_pool(name="p4w", bufs=1) as p4w,
        tc.tile_pool(name="p4s", bufs=3) as p4s,
        tc.tile_pool(name="p4ps", bufs=8, space="PSUM") as p4ps,
    ):
        yf_sb = [p4w.tile([128, T], BF16, tag=f"yf{f}", name=f"yf{f}") for f in range(ND)]
        for j in range(NB):
            for f in range(ND):
                eng = nc.sync if f % 2 == 0 else nc.scalar
                eng.dma_start(
                    yf_sb[f][:, 512 * j : 512 * (j + 1)],
                    ag_out[j].ap()[128 * f : 128 * (f + 1), :],
                )
        for t in range(NT):
            o_ps = p4ps.tile([128, QF], F32, tag="ops")
            for f in range(ND):
                nc.tensor.matmul(
                    o_ps[:], yf_sb[f][:, 128 * t : 128 * (t + 1)], wo_sb[f][:],
                    start=(f == 0), stop=(f == ND - 1),
                )
            o_sb = p4s.tile([128, QF], F32, tag="osb")
            nc.scalar.copy(o_sb[:], o_ps[:])
            nc.sync.dma_start(out_d[128 * t : 128 * (t + 1), :], o_sb[:])


_CACHED = None


def _build():
    global _CACHED
    if _CACHED is None:
        nc = bacc.Bacc(
            "TRN2", target_bir_lowering=False, debug=False, num_devices=NCORES
        )
        with tile.TileContext(nc) as tc:
            with ExitStack() as ctx:
                _emit(tc, ctx)
        nc.compile()
        _CACHED = nc
    return _CACHED


def _in_maps(x, ve_embed, Wq, Wk, Wv, Wo, q_gain):
    x = np.asarray(x, np.float32)
    ve_embed = np.asarray(ve_embed, np.float32)
    Wq = np.asarray(Wq, np.float32)
    Wk = np.asarray(Wk, np.float32)
    Wv = np.asarray(Wv, np.float32)
    Wo = np.asarray(Wo, np.float32)
    q_gain = np.asarray(q_gain, np.float32)

    tt = np.arange(T, dtype=np.float32)
    inv_freq = (
        1.0 / (ROPE_BASE ** (np.arange(0, HD, 2, dtype=np.float32) / np.float32(HD)))
    ).astype(np.float32)
    f = np.outer(tt, inv_freq)
    cs = np.concatenate(
        [np.tile(np.cos(f), (1, 4)), np.tile(np.sin(f), (1, 4))], axis=1
    ).astype(NPBF16)

    p = np.arange(128)[:, None]
    w = np.arange(128)[None, :]
    mask = (w >= p).astype(NPBF16)

    maps = []
    for core in range(NCORES):
        b, c = divmod(core, 4)
        qrows = slice(QF * c, QF * (c + 1))
        krows = slice(HD * c, HD * (c + 1))
        maps.append(
            {
                "xt": np.ascontiguousarray(x[b].T).astype(NPBF16),
                "wq": np.ascontiguousarray(Wq[qrows, :].T).astype(NPBF16),
                "wkv": np.ascontiguousarray(
                    np.concatenate([Wk[krows, :], Wv[krows, :]], axis=0).T
                ).astype(NPBF16),
                "ve": np.ascontiguousarray(ve_embed[b][:, krows]),
                "wo": np.ascontiguousarray(Wo[qrows, :].T).astype(NPBF16),
                "cs": cs,
                "qg": q_gain[None, HPC * c : HPC * (c + 1)].copy(),
                "mask": mask,
                "ident": np.eye(128, dtype=NPBF16),
            }
        )
    return maps


def _assemble(results):
    out = np.empty((B, T, D), np.float32)
    for core in range(NCORES):
        b, c = divmod(core, 4)
        out[b][:, QF * c : QF * (c + 1)] = results[core]["out"]
    return out


def run_traced(**inputs):
    nc = _build()
    maps = _in_maps(**inputs)
    r = run_bass_kernel_spmd(nc, maps, core_ids=list(range(NCORES)), trace=True)
    return _assemble(r.results), r


def kernel(**inputs):
    nc = _build()
    maps = _in_maps(**inputs)
    r = run_bass_kernel_spmd(nc, maps, core_ids=list(range(NCORES)))
    return _assemble(r.results)
